# revision 1
# baseline (speedup 1.0000x reference)
"""ConditionEmbedder kernel for 8 Trainium2 NeuronCores.

Math (train=0, unconditioned=0 path):
    drop = isnan(labels);  safe = where(drop, 0, labels)
    s    = softmax(safe[:,d,None]*w1[d] + b1[d], axis=-1)        # per (b, d)
    mlp  = s @ w2[d].T
    out  = sum_d where(drop, emb_w[d], mlp)                      # [B, H]

Device strategy (pure data parallel over batch, 8 cores):
    * softmax is computed WITHOUT a division:  s = exp(w1*x - lnZ(x)),
      where lnZ_d(x) = log sum_h exp(x*w1[d,h] + b1[d,h]) is approximated
      by a per-d degree-16 polynomial evaluated on the vector engine
      (b1 is folded into w2:  w2e = w2 * exp(b1)).
    * the [h, b] logits tile is produced by a K=6 matmul that simultaneously
      broadcasts x across 128 partitions, applies w1 (bf16 hi/lo split for
      fp32 accuracy), and adds -lnZ (the normalizer) and the -1e30 drop mask
      baked into the lncz rows.
    * scalar engine does one exp pass (the hard floor of this problem).
    * the H x H matmuls run as float32r (full PE rate) accumulating all 8 d
      plus the embedding-fallback term into one PSUM tile [k, b].
    * output is written transposed [H, B_core]; the host untransposes.
"""

import sys

import numpy as np

_B, _D, _H = 131072, 8, 128
_NCORES = 8
_BC = _B // _NCORES          # batch rows per core
_NSTRIPE = _BC // 512        # 512-row stripes per core
_DEG = 16                    # lnZ polynomial degree (zero-padded if fit is lower)


def _np_reference(labels, emb_w, w1, b1, w2, train, unconditioned):
    """Slow exact fallback for the train/unconditioned branches (uses jax to
    reproduce the reference PRNG streams)."""
    import jax
    import jax.numpy as jnp

    DROPOUT_PROB = 0.1
    labels = jnp.asarray(labels)
    if unconditioned:
        drop = jnp.ones(labels.shape, dtype=bool)
    else:
        drop = jnp.isnan(labels)
        if train:
            rkey = jax.random.fold_in(jax.random.key(0), 1)
            drop = drop | (jax.random.uniform(rkey, labels.shape) < DROPOUT_PROB)
    safe = jnp.where(drop, 0.0, labels)
    h1 = safe[:, :, None] * w1[None, :, :] + b1[None, :, :]
    s = jax.nn.softmax(h1, axis=-1)
    mlp = jnp.einsum('bdh,dkh->bdk', s, w2)
    emb = jnp.where(drop[:, :, None], emb_w[None, :, :], mlp)
    if train:
        nkey = jax.random.fold_in(jax.random.key(0), 2)
        emb = emb + jax.random.normal(nkey, emb.shape, dtype=emb.dtype)
    return np.asarray(emb.sum(axis=1))


def _fit_lnz_coeffs(w1, b1, S):
    """Per-d monomial coefficients c[d, 0..DEG] with poly(u) ~= -lnZ_d(S*u),
    validated against a simulated fp32 reversed-Horner evaluation."""
    import numpy.polynomial.chebyshev as CH

    G = 8193
    u = np.linspace(-1.0, 1.0, G)
    x = S * u
    lg = x[:, None, None] * w1[None].astype(np.float64) + b1[None].astype(np.float64)
    m = lg.max(-1)
    lnZ = m + np.log(np.exp(lg - m[..., None]).sum(-1))
    target = -lnZ  # [G, 8]

    uf = u.astype(np.float32)
    coeffs = np.zeros((_D, _DEG + 1), np.float64)
    for d in range(_D):
        best = None
        for deg in range(8, _DEG + 1):
            cf = CH.chebfit(u, target[:, d], deg)
            pc = CH.cheb2poly(cf)
            cc = np.zeros(_DEG + 1)
            cc[: len(pc)] = pc
            acc = np.zeros(G, np.float32)
            for k in range(_DEG, 0, -1):
                acc = ((acc + np.float32(cc[k])) * uf).astype(np.float32)
            err = np.abs((acc + np.float32(cc[0])).astype(np.float32) - target[:, d]).max()
            if best is None or err < best[0]:
                best = (err, cc)
        coeffs[d] = best[1]
    return coeffs.astype(np.float32)


def _bf16_split(a, bf16):
    hi = a.astype(bf16)
    lo = (a.astype(np.float32) - hi.astype(np.float32)).astype(bf16)
    return hi, lo


class _Builder:
    """Builds the per-core Bass program (identical on all cores; data differs)."""

    def __init__(self):
        sys.path.insert(0, '/opt/trn_rl_repo')
        import concourse.mybir as mybir
        from concourse import bass, tile
        from concourse.vector_clock import ScopedClock

        self.mybir = mybir
        self.bass = bass
        self.tile = tile
        self.ScopedClock = ScopedClock

    def make_tile_context(self, nc):
        mybir = self.mybir
        tile = self.tile
        ScopedClock = self.ScopedClock

        class PatchedTileContext(tile.TileContext):
            # walrus in this container rejects >1 sync-wait on the tail Drain
            # (setupSyncWait CTRL limit); spread the end-of-kernel waits
            # across single-wait SP nops instead.
            def _drain_and_barrier(self, tick_clock, wait_clock):
                nc_ = self.nc
                probe = nc_.sync.nop(nofuse=True)
                wait_clock.add_sem_waits(
                    probe.ins, ScopedClock({None: tick_clock.global_clock})
                )
                si = probe.ins.sync_info
                waits = list(si.on_wait) if si and si.on_wait else []
                if len(waits) > 1:
                    si.on_wait.clear()
                    si.on_wait.append(waits[0])
                    for w in waits[1:]:
                        n2 = nc_.sync.nop(nofuse=True)
                        s2 = n2.ins.sync_info
                        if s2 is None:
                            n2.ins.sync_info = mybir.SyncInfo(on_wait=[w], on_update=[])
                        else:
                            s2.on_wait.append(w)
                nc_.sync.drain()
                nc_.all_engine_barrier()
                assert self.sems is not None
                popped = nc_._tile_sem_poison_stack.pop()
                assert popped is self._sem_poison
                nc_.clear_and_free_semaphores(list(self.sems.allocated().values()))
                nc_.all_engine_barrier()

        return PatchedTileContext(nc)

    def build(self, inv_scale):
        mybir = self.mybir
        bass = self.bass
        dt = mybir.dt
        ALU = mybir.AluOpType
        F32, BF16, F16 = dt.float32, dt.bfloat16, dt.float16

        nc = bass.Bass(trn_type="TRN2", enable_partition_id=False)

        # ---- DRAM parameters ----
        # per-core labels, transposed-dense layout: row (d*16 + c) holds
        # labels[c*1024:(c+1)*1024, d]
        p_lab = nc.declare_dram_parameter("lab_td", [128, 1024], F32, isOutput=False)
        p_coef = nc.declare_dram_parameter("coeffs", [128, _DEG + 1], F32, isOutput=False)
        # affine stationary: rows 32i+0..5 = [w1h;w1h;w1l;w1l;1;1] for d=4g+i,
        # g selected by column block
        p_alhs = nc.declare_dram_parameter("aff_lhsT", [128, 256], BF16, isOutput=False)
        p_w2e = nc.declare_dram_parameter("w2eT", [128, 1024], BF16, isOutput=False)
        p_embw = nc.declare_dram_parameter("embw", [128, 128], BF16, isOutput=False)
        p_out = nc.declare_dram_parameter("outT", [128, _BC], F32, isOutput=True)

        from contextlib import ExitStack

        with self.make_tile_context(nc) as tc, ExitStack() as ctx:
            consts = ctx.enter_context(tc.tile_pool(name="consts", bufs=1))
            prep = ctx.enter_context(tc.tile_pool(name="prep", bufs=1))
            h1p = ctx.enter_context(tc.tile_pool(name="h1", bufs=2, space="PSUM"))
            pop = ctx.enter_context(tc.tile_pool(name="pout", bufs=2, space="PSUM"))
            sup = ctx.enter_context(tc.tile_pool(name="su", bufs=3))
            obp = ctx.enter_context(tc.tile_pool(name="ob", bufs=3))

            # ---- constants in ----
            t_coef = consts.tile([128, _DEG + 1], F32)
            nc.gpsimd.dma_start(t_coef[:], p_coef[:])
            t_alhs = consts.tile([128, 256], BF16)
            nc.gpsimd.dma_start(t_alhs[:], p_alhs[:])
            t_w2e = consts.tile([128, 1024], BF16)
            nc.gpsimd.dma_start(t_w2e[:], p_w2e[:])
            t_embw = consts.tile([128, 128], BF16)
            nc.gpsimd.dma_start(t_embw[:], p_embw[:])

            # ---- preamble: drop mask, safe labels, lncz polynomial ----
            t_x = prep.tile([128, 1024], F32)
            nc.gpsimd.dma_start(t_x[:], p_lab[:])

            t_eq = prep.tile([128, 1024], dt.uint8)
            nc.vector.tensor_tensor(t_eq[:], t_x[:], t_x[:], ALU.is_equal)

            t_safe = prep.tile([128, 1024], F32)
            nc.vector.memset(t_safe[:], 0.0)
            nc.vector.copy_predicated(t_safe[:], t_eq[:], t_x[:])

            t_u = prep.tile([128, 1024], F32)
            nc.vector.tensor_scalar_mul(t_u[:], t_safe[:], float(inv_scale))

            acc_a = prep.tile([128, 1024], F32)
            acc_b = prep.tile([128, 1024], F32)
            nc.vector.memset(acc_a[:], 0.0)
            cur, nxt = acc_a, acc_b
            for k in range(_DEG, 0, -1):
                nc.vector.scalar_tensor_tensor(
                    nxt[:], cur[:], t_coef[:, k:k + 1], t_u[:], ALU.add, ALU.mult
                )
                cur, nxt = nxt, cur
            t_lncz = prep.tile([128, 1024], F32)
            # lncz = poly + c0 where kept, -1e30 where dropped
            nc.vector.memset(t_lncz[:], -1.0e30)
            t_pl = nxt  # reuse the other ping-pong buffer
            nc.vector.tensor_scalar_add(t_pl[:], cur[:], t_coef[:, 0:1])
            nc.vector.copy_predicated(t_lncz[:], t_eq[:], t_pl[:])

            # bf16 hi/lo splits
            t_xh = prep.tile([128, 1024], BF16)
            nc.vector.tensor_copy(t_xh[:], t_safe[:])
            t_xl = prep.tile([128, 1024], BF16)
            nc.vector.tensor_tensor(t_xl[:], t_safe[:], t_xh[:], ALU.subtract)
            t_lh = prep.tile([128, 1024], BF16)
            nc.vector.tensor_copy(t_lh[:], t_lncz[:])
            t_ll = prep.tile([128, 1024], BF16)
            nc.vector.tensor_tensor(t_ll[:], t_lncz[:], t_lh[:], ALU.subtract)
            t_dropf = prep.tile([128, 1024], BF16)
            # drop = 1 - eq = eq * -1 + 1
            nc.vector.tensor_scalar(t_dropf[:], t_eq[:], -1.0, 1.0, ALU.mult, ALU.add)

            # ---- scatter into matmul-ready row layouts (SBUF->SBUF DMA) ----
            # Two column-halves per tensor so the first half's stripes can
            # start while the second half is still scattering.
            # afftile[g][h]: [128, BC/2] bf16, rows 32i+{0..5} =
            # [xh,xl,xh,xl,lh,ll] of d = 4g+i; batch b = c*1024 + h*512 + col.
            t_aff00 = consts.tile([128, _BC // 2], BF16)
            t_aff01 = consts.tile([128, _BC // 2], BF16)
            t_aff10 = consts.tile([128, _BC // 2], BF16)
            t_aff11 = consts.tile([128, _BC // 2], BF16)
            t_aff = [[t_aff00, t_aff01], [t_aff10, t_aff11]]
            t_emb0 = consts.tile([128, _BC // 2], BF16)
            t_emb1 = consts.tile([128, _BC // 2], BF16)
            t_emb = [t_emb0, t_emb1]
            rowsrc = [t_xh, t_xl, t_xh, t_xl, t_lh, t_ll]
            for h in range(2):
                csl = slice(512 * h, 512 * h + 512)
                for g in range(2):
                    for r, src in enumerate(rowsrc):
                        # in rows [64g:64g+64] iterate (i, c) lexicographic,
                        # matching out rows 32i+r (stride 32) x 16 col-blocks
                        nc.sync.dma_start(
                            t_aff[g][h][r:r + 97:32, :],
                            src[64 * g:64 * g + 64, csl],
                        )
                nc.sync.dma_start(t_emb[h][0:8, :], t_dropf[:, csl])

            # ---- main stripe loop (software-pipelined over d-groups) ----
            # Keep the PE stream dense (affine of group k+1 interleaves with
            # the exp-dependent mains of group k) so HAM stays at 2.4 GHz.
            Exp = mybir.ActivationFunctionType.Exp
            DGROUPS = [(0, 1, 2), (3, 4, 5), (6, 7)]
            NG = _NSTRIPE * 3

            def gslice(s):
                h, c = divmod(s, 16)
                return h, slice(512 * c, 512 * (c + 1))

            h1s = [None] * NG
            sus = [None] * NG

            def emit_aff(k):
                s, j = divmod(k, 3)
                h, sl = gslice(s)
                grp = DGROUPS[j]
                n = len(grp)
                h1 = h1p.tile([128, 512 * n], F32, tag="h1", name=f"h1_{k}")
                h1s[k] = h1
                for jj, d in enumerate(grp):
                    g, i = divmod(d, 4)
                    nc.tensor.matmul(
                        h1[:, 512 * jj:512 * (jj + 1)],
                        t_alhs[32 * i:32 * i + 6, 128 * g:128 * (g + 1)],
                        t_aff[g][h][32 * i:32 * i + 6, sl],
                        start=True, stop=True,
                        tile_position=(32 * i, 0),
                    )

            def emit_exp(k):
                s, j = divmod(k, 3)
                n = len(DGROUPS[j])
                su = sup.tile([128, 512 * n], BF16, tag="su", name=f"su_{k}")
                sus[k] = su
                nc.scalar.activation(su[:], h1s[k][:], Exp)

            pos = [None] * _NSTRIPE

            def emit_mains(k):
                s, j = divmod(k, 3)
                h, sl = gslice(s)
                grp = DGROUPS[j]
                if j == 0:
                    pos[s] = pop.tile([128, 512], F32, tag="po", name=f"po_{s}")
                po = pos[s]
                su = sus[k]
                for jj, d in enumerate(grp):
                    nc.tensor.matmul(
                        po[:],
                        t_w2e[:, 128 * d:128 * (d + 1)],
                        su[:, 512 * jj:512 * (jj + 1)],
                        start=(j == 0 and jj == 0), stop=False,
                    )
                if j == 2:
                    nc.tensor.matmul(
                        po[:], t_embw[0:8, :], t_emb[h][0:8, sl],
                        start=False, stop=True,
                    )
                    ob = obp.tile([128, 512], F32, tag="ob", name=f"ob_{s}")
                    nc.vector.tensor_copy(ob[:], po[:])
                    c = s % 16
                    osl = slice(1024 * c + 512 * (s // 16),
                                1024 * c + 512 * (s // 16) + 512)
                    nc.gpsimd.dma_start(p_out[:, osl], ob[:])

            # ---- HAM warm-up: ~5us of dense matmuls right before the
            # stripe loop (reads the scattered tile so it can't be scheduled
            # earlier). Gets the PE to 2.4 GHz; the steady-state gaps are too
            # short to re-throttle it.
            wtile = pop.tile([128, 512], F32, tag="po", name="warm")
            for it in range(16):
                nc.tensor.matmul(
                    wtile[:], t_alhs[0:6, 0:128], t_aff[0][0][0:6, 0:512],
                    start=True, stop=True, skip_group_check=True,
                    tile_position=(0, 0),
                )

            emit_aff(0)
            emit_exp(0)
            for k in range(NG):
                if k + 1 < NG:
                    emit_aff(k + 1)
                    emit_exp(k + 1)
                emit_mains(k)

        self._split_multi_waits(nc)
        return nc

    def _split_multi_waits(self, nc, maxw=1):
        """walrus in this container caps sync-waits per instruction at 2;
        move excess waits onto inserted same-engine NoOps."""
        mybir = self.mybir
        for f in nc.m.functions:
            for bb in f.blocks:
                new = []
                changed = False
                for ins in list(bb.instructions):
                    si = ins.sync_info
                    waits = list(si.on_wait) if si and si.on_wait else []
                    if len(waits) > maxw:
                        changed = True
                        extra, keep = waits[:-maxw], waits[-maxw:]
                        for j in range(0, len(extra), maxw):
                            new.append(mybir.InstNoOp(
                                name=f"{ins.name}_sw{j}", engine=ins.engine,
                                sync_info=mybir.SyncInfo(
                                    on_wait=list(extra[j:j + maxw]), on_update=[]),
                                text_hint="split_wait"))
                        si.on_wait.clear()
                        for w in keep:
                            si.on_wait.append(w)
                    new.append(ins)
                if changed:
                    bb.instructions = new


def _prepare_host(labels, emb_w, w1, b1, w2):
    import ml_dtypes
    bf16 = ml_dtypes.bfloat16

    S = float(max(6.0, np.nanmax(np.abs(labels)) * 1.02))
    coeffs = _fit_lnz_coeffs(w1, b1, S)  # [8, DEG+1] f32

    # coeff columns for the dense layout: partition p holds d = p // 16
    cc = np.zeros((128, _DEG + 1), np.float32)
    for p in range(128):
        cc[p] = coeffs[p // 16]

    w1h, w1l = _bf16_split(w1, bf16)            # [8, 128] each
    aff_lhsT = np.zeros((128, 256), bf16)
    ones = np.ones(_H, bf16)
    for d in range(_D):
        g, i = divmod(d, 4)
        rows = [w1h[d], w1h[d], w1l[d], w1l[d], ones, ones]
        for r, v in enumerate(rows):
            aff_lhsT[32 * i + r, 128 * g:128 * (g + 1)] = v

    w2e = (w2.astype(np.float64) * np.exp(b1.astype(np.float64))[:, None, :])
    w2eT = np.zeros((128, 1024), bf16)
    for d in range(_D):
        w2eT[:, 128 * d:128 * (d + 1)] = w2e[d].T.astype(bf16)

    embw = np.zeros((128, 128), bf16)
    embw[0:8] = emb_w.astype(bf16)

    # per-core transposed-dense labels: row 16*d + c = labels[c*1024:(c+1)*1024, d]
    lab_td = []
    for c in range(_NCORES):
        lc = labels[c * _BC:(c + 1) * _BC]               # [BC, 8]
        td = lc.reshape(16, 1024, 8).transpose(2, 0, 1).reshape(128, 1024)
        lab_td.append(np.ascontiguousarray(td, dtype=np.float32))

    const_map = {"coeffs": cc, "aff_lhsT": aff_lhsT, "w2eT": w2eT, "embw": embw}
    return S, lab_td, const_map


def _run_device(labels, emb_w, w1, b1, w2, trace=False):
    S, lab_td, const_map = _prepare_host(labels, emb_w, w1, b1, w2)
    builder = _Builder()
    nc = builder.build(1.0 / S)

    from concourse.bass_utils import run_bass_kernel_spmd
    in_maps = [{"lab_td": lab_td[c], **const_map} for c in range(_NCORES)]
    res = run_bass_kernel_spmd(
        nc, in_maps, list(range(_NCORES)), trace=trace
    )
    out = np.empty((_B, _H), np.float32)
    for c in range(_NCORES):
        out[c * _BC:(c + 1) * _BC] = res.results[c]["outT"].T
    return out, res


def kernel(labels, emb_w, w1, b1, w2, train, unconditioned):
    labels = np.asarray(labels)
    emb_w = np.asarray(emb_w, dtype=np.float32)
    w1 = np.asarray(w1, dtype=np.float32)
    b1 = np.asarray(b1, dtype=np.float32)
    w2 = np.asarray(w2, dtype=np.float32)
    if int(np.asarray(train)) or int(np.asarray(unconditioned)):
        return _np_reference(labels, emb_w, w1, b1, w2,
                             int(np.asarray(train)), int(np.asarray(unconditioned)))
    out, _ = _run_device(labels, emb_w, w1, b1, w2, trace=False)
    return out



# revision 4
# speedup vs baseline: 2.8391x; 2.8391x over previous
"""ConditionEmbedder kernel for 8 Trainium2 NeuronCores.

Math (train=0, unconditioned=0 path):
    drop = isnan(labels);  safe = where(drop, 0, labels)
    s    = softmax(safe[:,d,None]*w1[d] + b1[d], axis=-1)        # per (b, d)
    mlp  = s @ w2[d].T
    out  = sum_d where(drop, emb_w[d], mlp)                      # [B, H]

Key insight: the per-dim contribution f_d(x) = w2[d] @ softmax(x*w1[d]+b1[d])
is a smooth vector-valued function of ONE scalar.  On the host we fit each
f_d with a degree-15 Chebyshev series in the warped variable
v = tanh(alpha*x/S)/tanh(alpha), so the whole batch reduces to ONE K=128
matmul per 512-column stripe:

    out[k, b] = sum_{d, p=1..15} C[d,p,k] * T_p(v_{b,d})         (120 rows)
              + sum_d drop_{b,d} * (emb_w[d,k] - fhat_d(0,k))    (  8 rows)
              + sum_d C[d,0,k]                                   (bias, folded
                                            into the PSUM->SBUF copy engines)

Device pipeline (pure data parallel over batch, 8 cores):
    * scalar engine: one Tanh activation produces the warped variable.
    * vector engine: Chebyshev recurrence T_{p+1} = (2/tanh a) t0 T_p - T_{p-1}
      in bf16 (2x DVE rate).
    * SBUF->SBUF DMA scatters the 15 basis tiles + drop flags from the
      (d, chunk)-row layout into the 128-row moving layout.
    * tensor engine: 32 matmuls [128x128] @ [128x512] accumulate everything.
    * PSUM -> SBUF copies (split scalar/vector) add the constant term and
      downcast to fp16; output is written transposed [H, B_core] in fp16 and
      the host untransposes/upcasts.
"""

import sys

import numpy as np

_B, _D, _H = 131072, 8, 128
_NCORES = 8
_BC = _B // _NCORES          # batch rows per core
_P = 15                      # Chebyshev degree (rows 16d+1 .. 16d+15)
_ALPHA = 1.5                 # tanh warp strength


def _np_reference(labels, emb_w, w1, b1, w2, train, unconditioned):
    """Slow exact fallback for the train/unconditioned branches (uses jax to
    reproduce the reference PRNG streams)."""
    import jax
    import jax.numpy as jnp

    DROPOUT_PROB = 0.1
    labels = jnp.asarray(labels)
    if unconditioned:
        drop = jnp.ones(labels.shape, dtype=bool)
    else:
        drop = jnp.isnan(labels)
        if train:
            rkey = jax.random.fold_in(jax.random.key(0), 1)
            drop = drop | (jax.random.uniform(rkey, labels.shape) < DROPOUT_PROB)
    safe = jnp.where(drop, 0.0, labels)
    h1 = safe[:, :, None] * w1[None, :, :] + b1[None, :, :]
    s = jax.nn.softmax(h1, axis=-1)
    mlp = jnp.einsum('bdh,dkh->bdk', s, w2)
    emb = jnp.where(drop[:, :, None], emb_w[None, :, :], mlp)
    if train:
        nkey = jax.random.fold_in(jax.random.key(0), 2)
        emb = emb + jax.random.normal(nkey, emb.shape, dtype=emb.dtype)
    return np.asarray(emb.sum(axis=1))


def _fit_cheb(emb_w, w1, b1, w2, S):
    """Fit f_d(x) = w2[d] @ softmax(x*w1[d]+b1[d]) with Chebyshev series in
    v = tanh(alpha*x/S)/tanh(alpha).  Returns (chebT[128,128] bf16 stationary,
    bias[128] f32)."""
    import ml_dtypes
    import numpy.polynomial.chebyshev as CH
    bf16 = ml_dtypes.bfloat16

    G = 8193
    u = np.linspace(-1.0, 1.0, G)
    v = np.tanh(_ALPHA * u) / np.tanh(_ALPHA)
    C = np.zeros((_D, _P + 1, _H))
    for d in range(_D):
        lg = (S * u)[:, None] * w1[d][None, :].astype(np.float64) \
            + b1[d][None, :].astype(np.float64)
        m = lg.max(-1, keepdims=True)
        e = np.exp(lg - m)
        s = e / e.sum(-1, keepdims=True)
        C[d] = CH.chebfit(v, s @ w2[d].T.astype(np.float64), _P)

    Cb = C.astype(np.float32).astype(bf16)          # what the PE will see
    # f_d(0) as evaluated on-device (u=0 for dropped entries): T_p(0) cycle
    T0 = np.array([[1, 0, -1, 0][p % 4] for p in range(_P + 1)], np.float64)
    fhat0 = (Cb.astype(np.float64) * T0[None, :, None]).sum(1)      # [D, H]

    chebT = np.zeros((128, 128), bf16)
    for d in range(_D):
        chebT[16 * d, :] = (emb_w[d].astype(np.float64) - fhat0[d]).astype(bf16)
        for p in range(1, _P + 1):
            chebT[16 * d + p, :] = Cb[d, p]
    bias = C[:, 0, :].sum(0).astype(np.float32)     # [H]; added by copy engines
    return chebT, bias


class _Builder:
    """Builds the per-core Bass program (identical on all cores; data differs)."""

    def __init__(self):
        sys.path.insert(0, '/opt/trn_rl_repo')
        import concourse.mybir as mybir
        from concourse import bass, tile
        from concourse.vector_clock import ScopedClock

        self.mybir = mybir
        self.bass = bass
        self.tile = tile
        self.ScopedClock = ScopedClock

    def make_tile_context(self, nc):
        mybir = self.mybir
        tile = self.tile
        ScopedClock = self.ScopedClock

        class PatchedTileContext(tile.TileContext):
            # walrus in this container rejects >1 sync-wait on the tail Drain
            # (setupSyncWait CTRL limit); spread the end-of-kernel waits
            # across single-wait SP nops instead.
            def _drain_and_barrier(self, tick_clock, wait_clock):
                nc_ = self.nc
                probe = nc_.sync.nop(nofuse=True)
                wait_clock.add_sem_waits(
                    probe.ins, ScopedClock({None: tick_clock.global_clock})
                )
                si = probe.ins.sync_info
                waits = list(si.on_wait) if si and si.on_wait else []
                if len(waits) > 1:
                    si.on_wait.clear()
                    si.on_wait.append(waits[0])
                    for w in waits[1:]:
                        n2 = nc_.sync.nop(nofuse=True)
                        s2 = n2.ins.sync_info
                        if s2 is None:
                            n2.ins.sync_info = mybir.SyncInfo(on_wait=[w], on_update=[])
                        else:
                            s2.on_wait.append(w)
                nc_.sync.drain()
                nc_.all_engine_barrier()
                assert self.sems is not None
                popped = nc_._tile_sem_poison_stack.pop()
                assert popped is self._sem_poison
                nc_.clear_and_free_semaphores(list(self.sems.allocated().values()))
                nc_.all_engine_barrier()

        return PatchedTileContext(nc)

    def build(self, tanh_scale):
        """tanh_scale = alpha / S."""
        mybir = self.mybir
        bass = self.bass
        dt = mybir.dt
        ALU = mybir.AluOpType
        F32, BF16, F16 = dt.float32, dt.bfloat16, dt.float16
        Act = mybir.ActivationFunctionType

        ta = float(np.tanh(_ALPHA))

        nc = bass.Bass(trn_type="TRN2", enable_partition_id=False)

        # ---- DRAM parameters ----
        # per-core labels, transposed-dense layout: row (16d + c) holds
        # labels[c*1024:(c+1)*1024, d]
        p_lab = nc.declare_dram_parameter("lab_td", [128, 1024], F32, isOutput=False)
        p_cheb = nc.declare_dram_parameter("chebT", [128, 128], BF16, isOutput=False)
        p_bias = nc.declare_dram_parameter("biasv", [128, 1], F32, isOutput=False)
        p_out = nc.declare_dram_parameter("outT", [128, _BC], F16, isOutput=True)

        from contextlib import ExitStack

        with self.make_tile_context(nc) as tc, ExitStack() as ctx:
            consts = ctx.enter_context(tc.tile_pool(name="consts", bufs=1))
            prep = ctx.enter_context(tc.tile_pool(name="prep", bufs=1))
            pop = ctx.enter_context(tc.tile_pool(name="pout", bufs=6, space="PSUM"))
            obp = ctx.enter_context(tc.tile_pool(name="ob", bufs=6))

            # ---- constants in ----
            t_cheb = consts.tile([128, 128], BF16)
            nc.gpsimd.dma_start(t_cheb[:], p_cheb[:])
            t_bias = consts.tile([128, 1], F32)
            nc.gpsimd.dma_start(t_bias[:], p_bias[:])

            # ---- preamble: drop mask, safe labels, tanh warp ----
            t_x = prep.tile([128, 1024], F32)
            nc.gpsimd.dma_start(t_x[:], p_lab[:])

            t_eq = prep.tile([128, 1024], dt.uint8)
            nc.vector.tensor_tensor(t_eq[:], t_x[:], t_x[:], ALU.is_equal)

            t_safe = prep.tile([128, 1024], F32)
            nc.vector.memset(t_safe[:], 0.0)
            nc.vector.copy_predicated(t_safe[:], t_eq[:], t_x[:])

            t_mov = consts.tile([128, _BC], BF16)

            t_dropf = prep.tile([128, 1024], BF16)
            # drop = 1 - eq = eq * -1 + 1
            nc.vector.tensor_scalar(t_dropf[:], t_eq[:], -1.0, 1.0, ALU.mult, ALU.add)
            nc.sync.dma_start(t_mov[0:113:16, :], t_dropf[:])

            # scalar engine: t0 = tanh(safe * (alpha/S))
            t_t0 = prep.tile([128, 1024], F32)
            nc.scalar.activation(t_t0[:], t_safe[:], Act.Tanh, scale=float(tanh_scale))

            # v = t0/tanh(a)  (= T_1), wm = 2*t0/tanh(a) (recurrence multiplier)
            t_v = prep.tile([128, 1024], BF16)
            nc.vector.tensor_scalar_mul(t_v[:], t_t0[:], 1.0 / ta)
            t_wm = prep.tile([128, 1024], BF16)
            nc.vector.tensor_scalar_mul(t_wm[:], t_t0[:], 2.0 / ta)
            nc.sync.dma_start(t_mov[1:114:16, :], t_v[:])

            # Chebyshev recurrence in bf16: T_{p+1} = wm*T_p - T_{p-1}
            t_T = {1: t_v}
            tmp2 = prep.tile([128, 1024], BF16)
            nc.vector.tensor_tensor(tmp2[:], t_wm[:], t_v[:], ALU.mult)
            t_T[2] = prep.tile([128, 1024], BF16, name="t_T2")
            nc.vector.tensor_scalar_add(t_T[2][:], tmp2[:], -1.0)
            nc.sync.dma_start(t_mov[2:115:16, :], t_T[2][:])
            for p in range(3, _P + 1):
                tmp = prep.tile([128, 1024], BF16, name=f"tmp_{p}")
                nc.vector.tensor_tensor(tmp[:], t_wm[:], t_T[p - 1][:], ALU.mult)
                t_T[p] = prep.tile([128, 1024], BF16, name=f"t_T{p}")
                nc.vector.tensor_tensor(t_T[p][:], tmp[:], t_T[p - 2][:], ALU.subtract)
                nc.sync.dma_start(t_mov[p:p + 113:16, :], t_T[p][:])

            # ---- main loop: one K=128 matmul per 512-col stripe ----
            NS = _BC // 512
            for s in range(NS):
                sl = slice(512 * s, 512 * (s + 1))
                po = pop.tile([128, 512], F32, tag="po", name=f"po_{s}")
                nc.tensor.matmul(po[:], t_cheb[:], t_mov[:, sl],
                                 start=True, stop=True)
                ob = obp.tile([128, 512], F16, tag="ob", name=f"ob_{s}")
                # add the constant term during the PSUM->SBUF downcast; split
                # the copies between scalar (2/3) and vector (1/3) engines
                if s % 3 != 2:
                    nc.scalar.activation(ob[:], po[:], Act.Identity,
                                         bias=t_bias[:, 0:1])
                else:
                    nc.vector.tensor_scalar_add(ob[:], po[:], t_bias[:, 0:1])
                nc.gpsimd.dma_start(p_out[:, sl], ob[:])

        self._split_multi_waits(nc)
        return nc

    def _split_multi_waits(self, nc, maxw=1):
        """walrus in this container caps sync-waits per instruction at 2;
        move excess waits onto inserted same-engine NoOps."""
        mybir = self.mybir
        for f in nc.m.functions:
            for bb in f.blocks:
                new = []
                changed = False
                for ins in list(bb.instructions):
                    si = ins.sync_info
                    waits = list(si.on_wait) if si and si.on_wait else []
                    if len(waits) > maxw:
                        changed = True
                        extra, keep = waits[:-maxw], waits[-maxw:]
                        for j in range(0, len(extra), maxw):
                            new.append(mybir.InstNoOp(
                                name=f"{ins.name}_sw{j}", engine=ins.engine,
                                sync_info=mybir.SyncInfo(
                                    on_wait=list(extra[j:j + maxw]), on_update=[]),
                                text_hint="split_wait"))
                        si.on_wait.clear()
                        for w in keep:
                            si.on_wait.append(w)
                    new.append(ins)
                if changed:
                    bb.instructions = new


def _prepare_host(labels, emb_w, w1, b1, w2):
    mx = float(np.nanmax(np.abs(labels)))
    if not np.isfinite(mx) or mx <= 0:
        mx = 1.0
    S = 1.02 * mx
    chebT, bias = _fit_cheb(emb_w, w1, b1, w2, S)

    # per-core transposed-dense labels: row 16*d + c = labels[c*1024:(c+1)*1024, d]
    lab_td = []
    for c in range(_NCORES):
        lc = labels[c * _BC:(c + 1) * _BC]               # [BC, 8]
        td = lc.reshape(16, 1024, 8).transpose(2, 0, 1).reshape(128, 1024)
        lab_td.append(np.ascontiguousarray(td, dtype=np.float32))

    const_map = {"chebT": chebT, "biasv": bias.reshape(128, 1)}
    return S, lab_td, const_map


def _run_device(labels, emb_w, w1, b1, w2, trace=False):
    S, lab_td, const_map = _prepare_host(labels, emb_w, w1, b1, w2)
    builder = _Builder()
    nc = builder.build(_ALPHA / S)

    from concourse.bass_utils import run_bass_kernel_spmd
    in_maps = [{"lab_td": lab_td[c], **const_map} for c in range(_NCORES)]
    res = run_bass_kernel_spmd(
        nc, in_maps, list(range(_NCORES)), trace=trace
    )
    out = np.empty((_B, _H), np.float32)
    for c in range(_NCORES):
        out[c * _BC:(c + 1) * _BC] = res.results[c]["outT"].T.astype(np.float32)
    return out, res


def kernel(labels, emb_w, w1, b1, w2, train, unconditioned):
    labels = np.asarray(labels)
    emb_w = np.asarray(emb_w, dtype=np.float32)
    w1 = np.asarray(w1, dtype=np.float32)
    b1 = np.asarray(b1, dtype=np.float32)
    w2 = np.asarray(w2, dtype=np.float32)
    if int(np.asarray(train)) or int(np.asarray(unconditioned)):
        return _np_reference(labels, emb_w, w1, b1, w2,
                             int(np.asarray(train)), int(np.asarray(unconditioned)))
    out, _ = _run_device(labels, emb_w, w1, b1, w2, trace=False)
    return out
